# revision 1
# baseline (speedup 1.0000x reference)
"""Trainium2 Bass kernel for the attention-scoring module:

    out[b, s] = softmax_s( (enc[b] @ W.T + bias) @ h[b] )

Math: the bias term contributes a constant per (b, :) row, which cancels in
the softmax, and the two contractions reassociate:

    energies[b, s] = enc[b, s, :] . v[b]   with   v[b] = h[b] @ W

Sharding: data-parallel over batch — one batch per NeuronCore (B == 8 cores).

Per-core schedule (DMA engines are the serializing resource at ~360 GB/s):
  - W (1 MB) first, then enc[b] (16 MB) streamed as 64 uniform 128-row
    chunks.  A 128-row chunk transfers in ~728 ns while its fused
    multiply+row-sum DVE instruction takes ~640 ns, so the DVE tracks
    arrivals with no terminal backlog and the last energy column is ready
    ~1.5 us after the last HBM byte.
  - softmax shift comes from the first 32 columns mid-stream; exp of cols
    0..60 also runs mid-stream.  The tail after the final chunk is exp of 4
    columns + sum + reciprocal + two scales + one output DMA.
"""

from contextlib import ExitStack

import numpy as np

import concourse.tile as tile
from concourse import bacc, mybir
from concourse import bass_isa
from concourse.bass_utils import run_bass_kernel_spmd
from concourse.masks import make_identity

B, S, H = 8, 8192, 512
N_CORES = 8
P = 128
N_COLS = S // P  # 64 energy columns, E[p, t] = energy(s = t*128 + p)
F32 = mybir.dt.float32
ALU = mybir.AluOpType
ACTF = mybir.ActivationFunctionType
AXX = mybir.AxisListType.X

CHUNK_BUFS = 16
EC = 32   # softmax shift comes from the first 32 columns, mid-stream
MC = 63   # second exp/transpose stage covers cols EC..MC


def _build_kernel():
    nc = bacc.Bacc("TRN2", target_bir_lowering=False, debug=False)
    enc = nc.dram_tensor("enc", [S, H], F32, kind="ExternalInput")
    hvec = nc.dram_tensor("hvec", [1, H], F32, kind="ExternalInput")
    Wmat = nc.dram_tensor("W", [H, H], F32, kind="ExternalInput")
    out = nc.dram_tensor("out", [S], F32, kind="ExternalOutput")

    with ExitStack() as ctx:
        tc = ctx.enter_context(tile.TileContext(nc))
        consts = ctx.enter_context(tc.tile_pool(name="consts", bufs=1))
        small = ctx.enter_context(tc.tile_pool(name="small", bufs=1))
        chunks = ctx.enter_context(tc.tile_pool(name="chunks", bufs=CHUNK_BUFS))
        scratch = ctx.enter_context(tc.tile_pool(name="scratch", bufs=2))
        psum = ctx.enter_context(tc.tile_pool(name="psum", bufs=1, space="PSUM"))
        psum1 = ctx.enter_context(tc.tile_pool(name="psum1", bufs=1, space="PSUM"))

        # Constants.
        identity = consts.tile([P, P], F32)
        make_identity(nc, identity[:])
        one11 = consts.tile([1, 1], F32)
        nc.gpsimd.memset(one11[:], 1.0)
        ones_row = consts.tile([1, P], F32)
        nc.gpsimd.memset(ones_row[:], 1.0)
        neg_ones_row = consts.tile([1, P], F32)
        nc.gpsimd.memset(neg_ones_row[:], -1.0)

        # Output staging: probsT[t, p] = prob(s = t*128 + p).
        probsT_sb = small.tile([EC, P], F32)  # unscaled exp, rows 0..EC
        # `final` is a raw SBUF tensor, not a pool tile: the pool-close Drain
        # instructions then don't wait for the output DMA that reads it, so
        # the epilogue overlaps the store instead of trailing it.
        final_t = ctx.enter_context(nc.sbuf_tensor("final", [N_COLS, P], F32))
        final = final_t.ap()

        # ---- queue the input DMAs: hvec, W, then 64 uniform enc chunks ----
        # W halves first (big transfers keep the DMA bus busy while the SP
        # sequencer's issue pipeline ramps), then hvec, then the enc chunks.
        W_h = []
        for g in range(2):
            wh = small.tile([P, 2, H], F32, tag=f"wh{g}")
            W_h.append(wh)
            nc.sync.dma_start(
                wh[:],
                Wmat.ap()[g * 2 * P : (g + 1) * 2 * P, :].rearrange(
                    "(c p) h -> p c h", c=2, p=P
                ),
            )
        W_c = [W_h[0][:, 0, :], W_h[0][:, 1, :], W_h[1][:, 0, :], W_h[1][:, 1, :]]
        hrow = small.tile([1, H], F32)
        nc.sync.dma_start(hrow[:], hvec.ap())

        # Trigger the ACT exp table load at t=0 instead of in the tail.
        dummy_act = small.tile([1, 1], F32)
        nc.scalar.activation(dummy_act[:], one11[:], ACTF.Exp, bias=0.0, scale=1.0)

        # ---- v = h @ W, broadcast to all 128 partitions ----
        # PE p-state ramps LOW -> MID -> full over ~3us of continuous work;
        # a train of dummy transposes warms it so the fp32 v matmuls (4
        # cycles/row) run at full clock when W arrives.
        warm_tiles = []
        for i in range(24):
            wt = psum1.tile([P, P], F32, tag=f"htb{i % 2}")
            warm_tiles.append(wt)
            nc.tensor.transpose(wt[:], identity[:], identity[:])
        # Stage 1 fuses transpose+broadcast: hTb_c[m, n] = h[c*128+m] for all
        # n (a single matmul: hrow-chunk stationary x ones_row moving).
        # Stage 2 contracts: v_bc[m, n] = sum_c sum_p hTb_c[p, m] * W_c[p, n]
        # = sum_k h[k] W[k, n], identical on every output partition m.
        hT_sb = []
        for c in range(4):
            hT_ps = psum1.tile([P, P], F32, tag=f"htb{c % 2}")
            nc.tensor.matmul(
                hT_ps[:],
                hrow[:1, c * P : (c + 1) * P],
                ones_row[:],
                start=True,
                stop=True,
            )
            ht = small.tile([P, P], F32, tag=f"ht{c}")
            hT_sb.append(ht)
            nc.scalar.copy(ht[:], hT_ps[:])
        v_bc_ps = psum1.tile([P, H], F32, tag="vbc")
        for c in range(4):
            nc.tensor.matmul(
                v_bc_ps[:],
                hT_sb[c][:],
                W_c[c],
                start=(c == 0),
                stop=(c == 3),
            )
        v_sb = small.tile([P, H], F32)
        nc.scalar.copy(v_sb[:], v_bc_ps[:])

        # ---- main loop: stream enc, fused multiply+reduce on DVE ----
        E = small.tile([P, N_COLS], F32)
        E63z = small.tile([P, 1], F32)  # dedicated final column: its readers
        # and writer carry no waits on the other 63 columns' producers
        P_exp = small.tile([P, N_COLS + 1], F32)  # col 64 holds rs12
        rs1 = small.tile([P, 1], F32)
        negM_sb = small.tile([P, 1], F32)
        negM_ps = psum.tile([P, 1], F32, tag="colp")
        probsT_ps = psum.tile([EC, P], F32, tag="outp")
        probsT23_ps = psum.tile([N_COLS - EC, P], F32, tag="outp2")

        def emit_shift_chain():
            # Softmax shift from the first EC columns, computed mid-stream.
            # Any shift within ~80 of the true max keeps exp() finite, and
            # the shift cancels exactly in the final normalization.
            m_col = small.tile([P, 1], F32)
            nc.vector.tensor_reduce(m_col[:], E[:, :EC], axis=AXX, op=ALU.max)
            gmax = small.tile([1, 1], F32)
            nc.gpsimd.tensor_reduce(
                gmax[:], m_col[:], axis=mybir.AxisListType.C, op=ALU.max
            )
            # broadcast -shift to all partitions via matmul with -1s
            nc.tensor.matmul(
                negM_ps[:], neg_ones_row[:], gmax[:], start=True, stop=True
            )
            nc.scalar.copy(negM_sb[:], negM_ps[:])
            # exp + row-sum + transpose of the early columns, off critical path
            nc.scalar.activation(
                P_exp[:, :EC],
                E[:, :EC],
                ACTF.Exp,
                bias=negM_sb[:],
                scale=1.0,
                accum_out=rs1[:],
            )
            nc.tensor.transpose(probsT_ps[:], P_exp[:, :EC], identity[:])
            nc.scalar.copy(probsT_sb[:], probsT_ps[:])

        def emit_mid_chain():
            # exp of cols EC..MC; rs12 = rs1 + rs2 lands in P_exp[:, 64] so
            # the tail reduce covers it for free.  The transpose of cols
            # EC..64 happens once, in the tail.
            rs2 = small.tile([P, 1], F32)
            nc.scalar.activation(
                P_exp[:, EC:MC],
                E[:, EC:MC],
                ACTF.Exp,
                bias=negM_sb[:],
                scale=1.0,
                accum_out=rs2[:],
            )
            nc.vector.tensor_add(P_exp[:, N_COLS : N_COLS + 1], rs1[:], rs2[:])

        for t in range(N_COLS):
            ch = chunks.tile([P, H], F32, tag="chunk")
            nc.sync.dma_start(ch[:], enc.ap()[t * P : (t + 1) * P, :])
            # the final column gets a dedicated scratch tile so its DVE
            # dispatch carries no buffer-recycle wait on the critical path
            tag = "prodz" if t == N_COLS - 1 else "prod"
            prod = scratch.tile([P, H], F32, tag=tag)
            # fused multiply + free-dim sum in one DVE instruction
            nc.vector.scalar_tensor_tensor(
                out=prod[:],
                in0=ch[:],
                scalar=1.0,
                in1=v_sb[:],
                op0=ALU.bypass,
                op1=ALU.mult,
                accum_out=E63z[:] if t == N_COLS - 1 else E[:, t : t + 1],
            )
            if t + 1 == EC:
                emit_shift_chain()
            if t + 1 == MC:
                emit_mid_chain()

        # ---- softmax tail: only the final column remains ----
        nc.scalar.activation(
            P_exp[:, MC:N_COLS],
            E63z[:],
            ACTF.Exp,
            bias=negM_sb[:],
            scale=1.0,
        )
        # transpose of cols EC..64 (PE) runs parallel to the sum chain (DVE)
        nc.tensor.transpose(probsT23_ps[:], P_exp[:, EC:N_COLS], identity[:])
        # total row-sum: tail exps + rs12 in one reduce
        rs_tot = small.tile([P, 1], F32)
        nc.vector.tensor_reduce(
            rs_tot[:], P_exp[:, MC : N_COLS + 1], axis=AXX, op=ALU.add
        )
        # S on every partition via gpsimd all-reduce (cheaper than a PE
        # matmul + PSUM round-trip), then reciprocal on DVE
        S_bc = small.tile([P, 1], F32)
        nc.gpsimd.partition_all_reduce(S_bc[:], rs_tot[:], P, bass_isa.ReduceOp.add)
        SinvB = small.tile([N_COLS, 1], F32)
        nc.vector.reciprocal(SinvB[:], S_bc[:N_COLS, :])
        # scale (SinvB entries are identical, so base-0 slices are valid)
        nc.vector.tensor_scalar_mul(
            final[EC:, :], probsT23_ps[:], SinvB[: N_COLS - EC, :]
        )
        nc.vector.tensor_scalar_mul(final[:EC, :], probsT_sb[:], SinvB[:EC, :])
        nc.sync.dma_start(out.ap().rearrange("(t p) -> t p", p=P), final)

    nc.compile()
    return nc


_NC_CACHE = {}


def kernel(hidden, encoder_outputs, W, b):
    """Full (unsharded) inputs in, full output out; 8-core SPMD inside."""
    if "nc" not in _NC_CACHE:
        _NC_CACHE["nc"] = _build_kernel()
    nc = _NC_CACHE["nc"]

    hidden = np.asarray(hidden)
    enc = np.ascontiguousarray(np.asarray(encoder_outputs, dtype=np.float32))
    Wm = np.ascontiguousarray(np.asarray(W, dtype=np.float32))
    in_maps = [
        {
            "enc": enc[c],
            "hvec": np.ascontiguousarray(hidden[0, c][None, :].astype(np.float32)),
            "W": Wm,
        }
        for c in range(N_CORES)
    ]
    res = run_bass_kernel_spmd(nc, in_maps, core_ids=list(range(N_CORES)))
    return np.stack([res.results[c]["out"] for c in range(N_CORES)], axis=0).astype(
        np.float32
    )



# revision 6
# speedup vs baseline: 1.2192x; 1.2192x over previous
"""Trainium2 Bass kernel for the attention-scoring module:

    out[b, s] = softmax_s( (enc[b] @ W.T + bias) @ h[b] )

Math: the bias term contributes a constant per (b, :) row, which cancels in
the softmax, and the two contractions reassociate:

    energies[b, s] = enc[b, s, :] . v[b]   with   v[b] = h[b] @ W

Sharding: data-parallel over batch - one batch per NeuronCore (B == 8 cores).

This revision streams enc (and W, h) as float16: the softmax tolerates the
quantization (measured rel-l2 ~6e-4 on the harness inputs, vs the 2e-2
gate), and the serializing resource is the DMA bus, so halving the bytes
halves the stream time (16 MiB -> 8 MiB, ~46.6us -> ~23.3us of DMA busy).

At fp16 arrival rates (~364 ns per 128-row block) no single engine can keep
up with the dot products (the fused DVE multiply+row-sum runs at 1x,
~612 ns/block), so the 64 blocks are split across four paths:

  - 'P': the first N_PBLK blocks are DMA'd *transposed* (fp16 XBAR
         transpose, ~448 ns/block vs 364 regular) and reduced on the PE:
         matmul with the transposed v as stationary produces energies
         directly in [1, 512] row layout, one PSUM row per 4-block group.
         exp+sum is then one ACT op per group.  Costs ~84 ns/block extra
         DMA but nearly zero Vector/Act time.
  - 'D': DVE fused scalar_tensor_tensor multiply+row-sum  (~612 ns/block)
  - 'A': DVE tensor_tensor multiply in fp16 2x mode (~332 ns) + ACT
         Copy-activation with accum_out row-sum            (~810 ns/block)
  - 'G': GPSIMD tensor_tensor multiply (~1110 ns) + the same ACT reduce
         (the Pool engine cannot run the fused scalar_tensor_tensor).

Softmax is incremental: shift from the first SHIFT_C regular columns
mid-stream, staged exp+row-sum, PE transposes, and a short tail (exp of the
last columns + sum + reciprocal + scale + output DMAs).
"""

from contextlib import ExitStack

import numpy as np

import concourse.tile as tile
from concourse import bacc, mybir
from concourse import bass_isa
from concourse.bass_utils import run_bass_kernel_spmd
from concourse.masks import make_identity

B, S, H = 8, 8192, 512
N_CORES = 8
P = 128
N_COLS = S // P  # 64 blocks of 128 rows
F32 = mybir.dt.float32
F16 = mybir.dt.float16
ALU = mybir.AluOpType
ACTF = mybir.ActivationFunctionType
AXX = mybir.AxisListType.X

N_PGROUPS = 3                 # PE-path groups of 4 blocks each
N_PBLK = 4 * N_PGROUPS        # blocks 0..N_PBLK-1 go via the PE path
R0 = N_PBLK                   # first regular block
NR = N_COLS - N_PBLK          # number of regular blocks / E columns

SHIFT_C = 16  # shift max comes from the first 16 regular columns
EC = 32       # stage-1 exp/transpose boundary (32-aligned partition offsets)
MC = NR - 2   # second exp stage covers regular cols EC..MC

# Regular-block DMA grouping: big groups early, singles late.
DMA_GROUPS = [8] * 3 + ["P"] + [8] * 2 + [4] * 2 + [1] * 4

# Per-block engine costs (ns) for the offline greedy scheduler.
COST_DVE_FUSED = 612.0
COST_DVE_MULT = 332.0
COST_ACT_REDUCE = 810.0
COST_GP_MULT = 1110.0


def _assign_engines():
    """Greedy assignment of regular blocks to D/A/G by simulated finish
    time (the P blocks are fixed up front)."""
    t = 1870.0 + 3.0 + 4 * 364.0
    arrivals = {}
    blk = R0
    for g in DMA_GROUPS:
        if g == "P":
            t += N_PBLK * 448.0
            continue
        t += g * 364.0
        for i in range(g):
            arrivals[blk + i] = t + 900.0
        blk += g
    v_ready = 5400.0
    free = {"D": v_ready, "A": v_ready + 400.0, "G": v_ready}
    out = {}
    for b in range(R0, N_COLS):
        arr = arrivals[b]
        if b == N_COLS - 1:
            out[b] = "D"
            break
        cand = {
            "D": max(arr, free["D"]) + COST_DVE_FUSED,
            "A": max(max(arr, free["D"]) + COST_DVE_MULT, free["A"])
            + COST_ACT_REDUCE,
            "G": max(max(arr, free["G"]) + COST_GP_MULT, free["A"])
            + COST_ACT_REDUCE,
        }
        pick = min(cand, key=lambda k: cand[k])
        out[b] = pick
        if pick == "D":
            free["D"] = max(arr, free["D"]) + COST_DVE_FUSED
        elif pick == "A":
            free["D"] = max(arr, free["D"]) + COST_DVE_MULT
            free["A"] = cand["A"]
        else:
            free["G"] = max(arr, free["G"]) + COST_GP_MULT
            free["A"] = cand["G"]
    return out


def _build_kernel():
    nc = bacc.Bacc("TRN2", target_bir_lowering=False, debug=False)
    enc = nc.dram_tensor("enc", [S, H], F16, kind="ExternalInput")
    hvec = nc.dram_tensor("hvec", [1, H], F16, kind="ExternalInput")
    Wmat = nc.dram_tensor("W", [H, H], F16, kind="ExternalInput")
    out = nc.dram_tensor("out", [S], F32, kind="ExternalOutput")

    engine_of = _assign_engines()

    with ExitStack() as ctx:
        tc = ctx.enter_context(tile.TileContext(nc))
        consts = ctx.enter_context(tc.tile_pool(name="consts", bufs=1))
        small = ctx.enter_context(tc.tile_pool(name="small", bufs=1))
        psum = ctx.enter_context(tc.tile_pool(name="psum", bufs=1, space="PSUM"))

        # Big flat SBUF regions (raw tensors: no pool-close drain and no
        # buffer recycling, so block compute carries no WAR waits).
        enc_t = ctx.enter_context(nc.sbuf_tensor("enc_all", [P, NR, H], F16))
        enc_all = enc_t.ap()
        encT_t = ctx.enter_context(nc.sbuf_tensor("encT", [P, N_PBLK, H], F16))
        encT = encT_t.ap()
        prod_t = ctx.enter_context(nc.sbuf_tensor("prods", [P, NR, H], F16))
        prods = prod_t.ap()
        final_t = ctx.enter_context(nc.sbuf_tensor("final", [NR, P], F32))
        final = final_t.ap()
        # PE-path probs rows (partition 0): [1, group, 512] unscaled/scaled
        prow_t = ctx.enter_context(nc.sbuf_tensor("prow", [1, N_PGROUPS, H], F32))
        prow = prow_t.ap()
        prowS_t = ctx.enter_context(nc.sbuf_tensor("prowS", [1, N_PGROUPS, H], F32))
        prowS = prowS_t.ap()

        # Constants.
        identity = consts.tile([P, P], F32)
        make_identity(nc, identity[:])
        one11 = consts.tile([1, 1], F32)
        nc.gpsimd.memset(one11[:], 1.0)
        ones_row16 = consts.tile([1, P], F16)
        nc.gpsimd.memset(ones_row16[:], 1.0)

        # ---- input DMA queue: hvec, W, regular groups with the P-path
        # transposes woven in mid-stream ----
        hrow = small.tile([1, H], F16)
        nc.sync.dma_start(hrow[:], hvec.ap())
        W_sb = small.tile([P, 4, H], F16)
        for c in range(4):
            nc.sync.dma_start(W_sb[:, c, :], Wmat.ap()[c * P : (c + 1) * P, :])
        blk = R0
        for g in DMA_GROUPS:
            if g == "P":
                # XBAR-transposed loads: group pg covers enc rows
                # [512*pg, 512*(pg+1)), h-quarter q -> encT[:, 4*pg+q, :]
                for pg in range(N_PGROUPS):
                    for q in range(4):
                        nc.sync.dma_start_transpose(
                            encT[:, 4 * pg + q, :],
                            enc.ap()[
                                512 * pg : 512 * (pg + 1),
                                128 * q : 128 * (q + 1),
                            ],
                        )
                continue
            nc.sync.dma_start(
                enc_all[:, blk - R0 : blk - R0 + g, :],
                enc.ap()[blk * P : (blk + g) * P, :].rearrange(
                    "(c p) h -> p c h", c=g, p=P
                ),
            )
            blk += g

        # Trigger the ACT exp table load at t=0 instead of in the tail.
        dummy_act = small.tile([1, 1], F32)
        nc.scalar.activation(dummy_act[:], one11[:], ACTF.Exp, bias=0.0, scale=1.0)

        # PE p-state warm-up.
        warm = psum.tile([P, P], F32, tag="pa")
        for i in range(12):
            nc.tensor.transpose(warm[:], identity[:], identity[:])

        # ---- v = h @ W (broadcast along free dim) and vT (along partitions) --
        hT_sb = []
        for c in range(4):
            hT_ps = psum.tile([P, P], F32, tag=f"p{'ab'[c % 2]}")
            nc.tensor.matmul(
                hT_ps[:],
                hrow[:1, c * P : (c + 1) * P],
                ones_row16[:],
                start=True,
                stop=True,
            )
            ht = small.tile([P, P], F16, tag=f"ht{c}")
            hT_sb.append(ht)
            nc.scalar.copy(ht[:], hT_ps[:])
        v_bc_ps = psum.tile([P, H], F32, tag="vbc")
        for c in range(4):
            nc.tensor.matmul(
                v_bc_ps[:],
                hT_sb[c][:],
                W_sb[:, c, :],
                start=(c == 0),
                stop=(c == 3),
            )
        v_sb = small.tile([P, H], F16)
        nc.scalar.copy(v_sb[:], v_bc_ps[:])
        # vT[p, c] = v[c*128+p], for the PE path stationary.
        vT_ps = psum.tile([P, 4], F32, tag="pa")
        for c in range(4):
            for kc in range(4):
                nc.tensor.matmul(
                    vT_ps[:, c : c + 1],
                    W_sb[:, kc, c * P : (c + 1) * P],
                    hT_sb[kc][:, :1],
                    start=(kc == 0),
                    stop=(kc == 3),
                )
        vT_sb = small.tile([P, 4], F16)
        nc.scalar.copy(vT_sb[:], vT_ps[:])

        # ---- main loop ----
        E = small.tile([P, NR], F32)
        E63z = small.tile([P, 1], F32)  # dedicated final column
        P_exp = small.tile([P, NR + 1], F32)  # col NR holds rs12
        rs1 = small.tile([P, 1], F32)
        negM_sb = small.tile([P, 1], F32)
        probsT_sb = small.tile([EC, P], F32)
        probsT_ps = psum.tile([EC, P], F32, tag="pb")
        probsT23_ps = psum.tile([NR - EC, P], F32, tag="vbc")
        prow_ps = []
        rrow = []
        for g in range(N_PGROUPS):
            prow_ps_g = psum.tile([1, H], F32, tag=f"prow{g}", name=f"prow_ps{g}")
            prow_ps.append(prow_ps_g)
            rrow_g = small.tile([1, 1], F32, tag=f"rr{g}", name=f"rrow{g}")
            rrow.append(rrow_g)
        rr_sum = small.tile([1, 1], F32)

        def emit_shift_chain():
            m_col = small.tile([P, 1], F32)
            nc.vector.tensor_reduce(
                m_col[:], E[:, :SHIFT_C], axis=AXX, op=ALU.max
            )
            M_bc = small.tile([P, 1], F32)
            nc.gpsimd.partition_all_reduce(
                M_bc[:], m_col[:], P, bass_isa.ReduceOp.max
            )
            nc.vector.tensor_scalar_mul(negM_sb[:], M_bc[:], -1.0)

        def emit_stage1_chain():
            nc.scalar.activation(
                P_exp[:, :EC],
                E[:, :EC],
                ACTF.Exp,
                bias=negM_sb[:],
                scale=1.0,
                accum_out=rs1[:],
            )
            nc.tensor.transpose(probsT_ps[:], P_exp[:, :EC], identity[:])
            nc.scalar.copy(probsT_sb[:], probsT_ps[:])

        def emit_mid_chain():
            rs2 = small.tile([P, 1], F32)
            nc.scalar.activation(
                P_exp[:, EC:MC],
                E[:, EC:MC],
                ACTF.Exp,
                bias=negM_sb[:],
                scale=1.0,
                accum_out=rs2[:],
            )
            nc.vector.tensor_add(P_exp[:, NR : NR + 1], rs1[:], rs2[:])

        def emit_pgroup_mm(pg):
            for q in range(4):
                nc.tensor.matmul(
                    prow_ps[pg][:],
                    vT_sb[:, q : q + 1],
                    encT[:, 4 * pg + q, :],
                    start=(q == 0),
                    stop=(q == 3),
                )

        def emit_pgroup_exp(pg):
            nc.scalar.activation(
                prow[:, pg, :],
                prow_ps[pg][:],
                ACTF.Exp,
                bias=negM_sb[:1, :],
                scale=1.0,
                accum_out=rrow[pg][:],
            )

        for t in range(R0, N_COLS):
            col = t - R0
            ch = enc_all[:, col, :]
            eng = engine_of[t]
            acc = E63z[:] if t == N_COLS - 1 else E[:, col : col + 1]
            pr = prods[:, col, :]
            if eng == "D":
                nc.vector.scalar_tensor_tensor(
                    out=pr, in0=ch, scalar=1.0, in1=v_sb[:],
                    op0=ALU.bypass, op1=ALU.mult, accum_out=acc,
                )
            elif eng == "G":
                nc.gpsimd.tensor_tensor(pr, ch, v_sb[:], op=ALU.mult)
                nc.scalar.activation(
                    pr, pr, ACTF.Copy, bias=0.0, scale=1.0, accum_out=acc,
                )
            else:  # 'A': fp16 2x multiply on DVE, reduce on ACT
                nc.vector.tensor_tensor(pr, ch, v_sb[:], op=ALU.mult)
                nc.scalar.activation(
                    pr, pr, ACTF.Copy, bias=0.0, scale=1.0, accum_out=acc,
                )
            if col + 1 == SHIFT_C:
                emit_shift_chain()
            if col + 1 == EC:
                emit_stage1_chain()
            if col + 1 == MC:
                emit_mid_chain()
            # weave the PE-path matmuls + exps into the stream
            if col == 20:
                emit_pgroup_mm(0)
            if col == 24:
                emit_pgroup_mm(1)
                emit_pgroup_exp(0)
            if col == 28:
                emit_pgroup_mm(2)
                emit_pgroup_exp(1)
            if col == 32:
                emit_pgroup_exp(2)
            if col == 36:
                # rr_sum = rrow0 + rrow1 (+ rrow2), folded ahead of the tail
                nc.vector.tensor_add(rr_sum[:], rrow[0][:], rrow[1][:])
                nc.vector.tensor_add(rr_sum[:], rr_sum[:], rrow[2][:])

        # ---- softmax tail ----
        nc.scalar.activation(
            P_exp[:, MC : NR - 1],
            E[:, MC : NR - 1],
            ACTF.Exp,
            bias=negM_sb[:],
            scale=1.0,
        )
        nc.scalar.activation(
            P_exp[:, NR - 1 : NR],
            E63z[:],
            ACTF.Exp,
            bias=negM_sb[:],
            scale=1.0,
        )
        nc.tensor.transpose(probsT23_ps[:], P_exp[:, EC:NR], identity[:])
        rs_tot = small.tile([P, 1], F32)
        nc.vector.tensor_reduce(
            rs_tot[:], P_exp[:, MC : NR + 1], axis=AXX, op=ALU.add
        )
        # fold the PE-path group sums into partition 0 before the all-reduce
        nc.vector.tensor_add(rs_tot[:1, :], rs_tot[:1, :], rr_sum[:])
        S_bc = small.tile([P, 1], F32)
        nc.gpsimd.partition_all_reduce(S_bc[:], rs_tot[:], P, bass_isa.ReduceOp.add)
        SinvB = small.tile([NR, 1], F32)
        nc.vector.reciprocal(SinvB[:], S_bc[:NR, :])
        # scale the PE-path rows on three engines in parallel
        nc.vector.tensor_scalar_mul(prowS[:, 0, :], prow[:, 0, :], SinvB[:1, :])
        nc.scalar.activation(
            prowS[:, 1, :], prow[:, 1, :], ACTF.Copy, bias=0.0, scale=SinvB[:1, :]
        )
        nc.gpsimd.tensor_scalar_mul(prowS[:, 2, :], prow[:, 2, :], SinvB[:1, :])
        # scale the regular columns (SinvB entries identical -> base-0 slices)
        nc.vector.tensor_scalar_mul(
            final[EC:, :], probsT23_ps[:], SinvB[: NR - EC, :]
        )
        nc.vector.tensor_scalar_mul(final[:EC, :], probsT_sb[:], SinvB[:EC, :])
        nc.sync.dma_start(
            out.ap()[: N_PBLK * P].rearrange("(a s) -> a s", a=1), prowS[:1]
        )
        nc.sync.dma_start(
            out.ap()[N_PBLK * P :].rearrange("(t p) -> t p", p=P), final
        )

    nc.compile()
    return nc


_NC_CACHE = {}


def kernel(hidden, encoder_outputs, W, b):
    """Full (unsharded) inputs in, full output out; 8-core SPMD inside."""
    if "nc" not in _NC_CACHE:
        _NC_CACHE["nc"] = _build_kernel()
    nc = _NC_CACHE["nc"]

    hidden = np.asarray(hidden)
    enc16 = np.ascontiguousarray(np.asarray(encoder_outputs).astype(np.float16))
    W16 = np.ascontiguousarray(np.asarray(W).astype(np.float16))
    in_maps = [
        {
            "enc": enc16[c],
            "hvec": np.ascontiguousarray(
                hidden[0, c][None, :].astype(np.float16)
            ),
            "W": W16,
        }
        for c in range(N_CORES)
    ]
    res = run_bass_kernel_spmd(nc, in_maps, core_ids=list(range(N_CORES)))
    return np.stack([res.results[c]["out"] for c in range(N_CORES)], axis=0).astype(
        np.float32
    )


# revision 8
# speedup vs baseline: 1.3917x; 1.1415x over previous
"""Trainium2 Bass kernel for the attention-scoring module:

    out[b, s] = softmax_s( (enc[b] @ W.T + bias) @ h[b] )

Math: the bias term contributes a constant per (b, :) row, which cancels in
the softmax, and the two contractions reassociate:

    energies[b, s] = enc[b, s, :] . v[b]   with   v[b] = h[b] @ W

Sharding: data-parallel over batch - one batch per NeuronCore (B == 8 cores).

This revision streams enc (and W, h) as float16: the softmax tolerates the
quantization (measured rel-l2 ~6e-4 on the harness inputs, vs the 2e-2
gate), and the serializing resource is the DMA bus, so halving the bytes
halves the stream time (16 MiB -> 8 MiB, ~46.6us -> ~23.3us of DMA busy).

At fp16 arrival rates (~364 ns per 128-row block) no single engine keeps up
with the dot products (the fused DVE multiply+row-sum runs at 1x,
~612 ns/block), so the 64 blocks are split across three paths:

  - 'D': DVE fused scalar_tensor_tensor multiply+row-sum  (~612 ns/block)
  - 'A': DVE tensor_tensor multiply in fp16 2x mode (~332 ns) + ACT
         Copy-activation with accum_out row-sum            (~810 ns/block)
  - 'G': GPSIMD tensor_tensor multiply (~1110 ns) + the same ACT reduce
         (the Pool engine cannot run the fused scalar_tensor_tensor, and
         XBAR-transposed loads for a PE path serialize against regular
         DMAs, so both alternatives lose).

Softmax is incremental: shift from the first SHIFT_C columns mid-stream,
staged exp+row-sum, PE transposes of the prob columns, and a short tail
(exp of the last columns + sum + reciprocal + scale + one output DMA).
"""

from contextlib import ExitStack

import numpy as np

import concourse.tile as tile
from concourse import bacc, mybir
from concourse import bass_isa
from concourse.bass_utils import run_bass_kernel_spmd
from concourse.masks import make_identity

B, S, H = 8, 8192, 512
N_CORES = 8
P = 128
N_COLS = S // P  # 64 energy columns, E[p, t] = energy(s = t*128 + p)
F32 = mybir.dt.float32
F16 = mybir.dt.float16
ALU = mybir.AluOpType
ACTF = mybir.ActivationFunctionType
AXX = mybir.AxisListType.X

SHIFT_C = 16  # softmax shift comes from the first 16 columns, mid-stream
EC = 32       # stage-1 exp/transpose boundary (32-aligned partition offsets)
MC = 62       # second exp stage covers cols EC..MC

# enc DMA grouping: big groups early, singles late so the tail only waits
# on one 128-row block.
DMA_GROUPS = [8] * 6 + [4] * 2 + [2] * 2 + [1] * 4

# Per-block engine costs (ns) for the offline greedy scheduler.
COST_DVE_FUSED = 612.0
COST_DVE_MULT = 332.0
COST_ACT_REDUCE = 810.0
COST_GP_MULT = 1110.0


def _assign_engines():
    """Greedy assignment of blocks to D/A/G by simulated finish time."""
    t = 1970.0 + 3.0 + 4 * 364.0
    arrivals = []
    for g in DMA_GROUPS:
        t += g * 364.0
        arrivals += [t + 900.0] * g
    v_ready = 5700.0
    free = {"D": v_ready, "A": v_ready + 400.0, "G": v_ready}
    out = []
    for b in range(N_COLS):
        arr = arrivals[b]
        if b == N_COLS - 1:
            out.append("D")
            break
        cand = {
            "D": max(arr, free["D"]) + COST_DVE_FUSED,
            "A": max(max(arr, free["D"]) + COST_DVE_MULT, free["A"])
            + COST_ACT_REDUCE,
            "G": max(max(arr, free["G"]) + COST_GP_MULT, free["A"])
            + COST_ACT_REDUCE,
        }
        # mid-stream exp stages eat ACT time; model them as load bumps
        pick = min(cand, key=lambda k: cand[k])
        out.append(pick)
        if pick == "D":
            free["D"] = max(arr, free["D"]) + COST_DVE_FUSED
        elif pick == "A":
            free["D"] = max(arr, free["D"]) + COST_DVE_MULT
            free["A"] = cand["A"]
        else:
            free["G"] = max(arr, free["G"]) + COST_GP_MULT
            free["A"] = cand["G"]
        if b + 1 == SHIFT_C:
            free["A"] += 400.0  # stage-1 exp + accum
        if b + 1 == MC:
            free["A"] += 450.0  # stage-2 exp + accum
    return out


def _build_kernel():
    nc = bacc.Bacc("TRN2", target_bir_lowering=False, debug=False)
    enc = nc.dram_tensor("enc", [S, H], F16, kind="ExternalInput")
    hvec = nc.dram_tensor("hvec", [1, H], F16, kind="ExternalInput")
    Wmat = nc.dram_tensor("W", [H, H], F16, kind="ExternalInput")
    out = nc.dram_tensor("out", [S], F32, kind="ExternalOutput")

    engine_of = _assign_engines()

    with ExitStack() as ctx:
        tc = ctx.enter_context(tile.TileContext(nc))
        consts = ctx.enter_context(tc.tile_pool(name="consts", bufs=1))
        small = ctx.enter_context(tc.tile_pool(name="small", bufs=1))
        psum = ctx.enter_context(tc.tile_pool(name="psum", bufs=1, space="PSUM"))

        # Big flat SBUF regions (raw tensors: no pool-close drain and no
        # buffer recycling, so block compute carries no WAR waits).
        enc_t = ctx.enter_context(nc.sbuf_tensor("enc_all", [P, N_COLS, H], F16))
        enc_all = enc_t.ap()
        prod_t = ctx.enter_context(nc.sbuf_tensor("prods", [P, N_COLS, H], F16))
        prods = prod_t.ap()
        final_t = ctx.enter_context(nc.sbuf_tensor("final", [N_COLS, P], F32))
        final = final_t.ap()

        # Constants.
        identity = consts.tile([P, P], F32)
        make_identity(nc, identity[:])
        one11 = consts.tile([1, 1], F32)
        nc.gpsimd.memset(one11[:], 1.0)
        ones_row16 = consts.tile([1, P], F16)
        nc.gpsimd.memset(ones_row16[:], 1.0)

        # ---- input DMA queue: hvec, W (4 chunks), then enc groups ----
        hrow = small.tile([1, H], F16)
        nc.sync.dma_start(hrow[:], hvec.ap())
        W_sb = small.tile([P, 4, H], F16)
        for c in range(4):
            nc.sync.dma_start(W_sb[:, c, :], Wmat.ap()[c * P : (c + 1) * P, :])
        blk = 0
        for g in DMA_GROUPS:
            nc.sync.dma_start(
                enc_all[:, blk : blk + g, :],
                enc.ap()[blk * P : (blk + g) * P, :].rearrange(
                    "(c p) h -> p c h", c=g, p=P
                ),
            )
            blk += g

        # Trigger the ACT exp table load at t=0 instead of in the tail.
        dummy_act = small.tile([1, 1], F32)
        nc.scalar.activation(dummy_act[:], one11[:], ACTF.Exp, bias=0.0, scale=1.0)

        # PE p-state warm-up: keep the PE continuously busy until hrow
        # lands so the v matmuls run at a ramped clock.
        warm = psum.tile([P, P], F32, tag="pwarm")
        for i in range(16):
            nc.tensor.transpose(warm[:], identity[:], identity[:])

        # ---- v = h @ W, broadcast to all 128 partitions (fp16 pipeline) ----
        # Stage 1: hTb_c[m, n] = h[c*128+m] for all n (hrow-chunk stationary
        # x ones_row moving); 4 distinct PSUM tags so nothing ping-pongs.
        # Copies PSUM->SBUF alternate ACT/DVE to halve the serial chain.
        hT_sb = []
        for c in range(4):
            hT_ps = psum.tile([P, P], F32, tag=f"ph{c}", name=f"hT_ps{c}")
            nc.tensor.matmul(
                hT_ps[:],
                hrow[:1, c * P : (c + 1) * P],
                ones_row16[:],
                start=True,
                stop=True,
            )
            ht = small.tile([P, P], F16, tag=f"ht{c}", name=f"ht{c}")
            hT_sb.append(ht)
            if c % 2 == 0:
                nc.scalar.copy(ht[:], hT_ps[:])
            else:
                nc.vector.tensor_copy(ht[:], hT_ps[:])
        v_bc_ps = psum.tile([P, H], F32, tag="vbc")
        for c in range(4):
            nc.tensor.matmul(
                v_bc_ps[:],
                hT_sb[c][:],
                W_sb[:, c, :],
                start=(c == 0),
                stop=(c == 3),
            )
        # v copy split across ACT and DVE so the consumers start sooner.
        v_sb = small.tile([P, H], F16)
        nc.scalar.copy(v_sb[:, : H // 2], v_bc_ps[:, : H // 2])
        nc.vector.tensor_copy(v_sb[:, H // 2 :], v_bc_ps[:, H // 2 :])

        # ---- main loop: stream enc, dot products split across 3 engines ----
        E = small.tile([P, N_COLS], F32)
        E63z = small.tile([P, 1], F32)  # dedicated final column
        P_exp = small.tile([P, N_COLS + 1], F32)  # col 64 holds rs12
        rs1 = small.tile([P, 1], F32)
        negM_sb = small.tile([P, 1], F32)
        probsT_ps = psum.tile([EC, P], F32, tag="ph0")
        probsT23_ps = psum.tile([N_COLS - EC, P], F32, tag="vbc")

        def emit_shift_chain():
            m_col = small.tile([P, 1], F32)
            nc.vector.tensor_reduce(
                m_col[:], E[:, :SHIFT_C], axis=AXX, op=ALU.max
            )
            M_bc = small.tile([P, 1], F32)
            nc.gpsimd.partition_all_reduce(
                M_bc[:], m_col[:], P, bass_isa.ReduceOp.max
            )
            nc.vector.tensor_scalar_mul(negM_sb[:], M_bc[:], -1.0)

        def emit_stage1_chain():
            nc.scalar.activation(
                P_exp[:, :EC],
                E[:, :EC],
                ACTF.Exp,
                bias=negM_sb[:],
                scale=1.0,
                accum_out=rs1[:],
            )
            nc.tensor.transpose(probsT_ps[:], P_exp[:, :EC], identity[:])

        def emit_mid_chain():
            rs2 = small.tile([P, 1], F32)
            nc.scalar.activation(
                P_exp[:, EC:MC],
                E[:, EC:MC],
                ACTF.Exp,
                bias=negM_sb[:],
                scale=1.0,
                accum_out=rs2[:],
            )
            nc.vector.tensor_add(P_exp[:, N_COLS : N_COLS + 1], rs1[:], rs2[:])

        for t in range(N_COLS):
            ch = enc_all[:, t, :]
            eng = engine_of[t]
            acc = E63z[:] if t == N_COLS - 1 else E[:, t : t + 1]
            pr = prods[:, t, :]
            if eng == "D":
                nc.vector.scalar_tensor_tensor(
                    out=pr, in0=ch, scalar=1.0, in1=v_sb[:],
                    op0=ALU.bypass, op1=ALU.mult, accum_out=acc,
                )
            elif eng == "G":
                nc.gpsimd.tensor_tensor(pr, ch, v_sb[:], op=ALU.mult)
                nc.scalar.activation(
                    pr, pr, ACTF.Copy, bias=0.0, scale=1.0, accum_out=acc,
                )
            else:  # 'A': fp16 2x multiply on DVE, reduce on ACT
                nc.vector.tensor_tensor(pr, ch, v_sb[:], op=ALU.mult)
                nc.scalar.activation(
                    pr, pr, ACTF.Copy, bias=0.0, scale=1.0, accum_out=acc,
                )
            if t + 1 == SHIFT_C:
                emit_shift_chain()
            if t + 1 == EC:
                emit_stage1_chain()
            if t + 1 == MC:
                emit_mid_chain()

        # ---- softmax tail: columns MC..63 ----
        nc.scalar.activation(
            P_exp[:, MC : N_COLS - 1],
            E[:, MC : N_COLS - 1],
            ACTF.Exp,
            bias=negM_sb[:],
            scale=1.0,
        )
        nc.scalar.activation(
            P_exp[:, N_COLS - 1 : N_COLS],
            E63z[:],
            ACTF.Exp,
            bias=negM_sb[:],
            scale=1.0,
        )
        nc.tensor.transpose(probsT23_ps[:], P_exp[:, EC:N_COLS], identity[:])
        rs_tot = small.tile([P, 1], F32)
        nc.vector.tensor_reduce(
            rs_tot[:], P_exp[:, MC : N_COLS + 1], axis=AXX, op=ALU.add
        )
        S_bc = small.tile([P, 1], F32)
        nc.gpsimd.partition_all_reduce(S_bc[:], rs_tot[:], P, bass_isa.ReduceOp.add)
        SinvB = small.tile([N_COLS, 1], F32)
        nc.vector.reciprocal(SinvB[:], S_bc[:N_COLS, :])
        # scale straight out of PSUM (SinvB entries identical -> base-0 ok)
        nc.vector.tensor_scalar_mul(
            final[EC:, :], probsT23_ps[:], SinvB[: N_COLS - EC, :]
        )
        nc.vector.tensor_scalar_mul(final[:EC, :], probsT_ps[:], SinvB[:EC, :])
        nc.sync.dma_start(out.ap().rearrange("(t p) -> t p", p=P), final)

    nc.compile()
    return nc


_NC_CACHE = {}


def kernel(hidden, encoder_outputs, W, b):
    """Full (unsharded) inputs in, full output out; 8-core SPMD inside."""
    if "nc" not in _NC_CACHE:
        _NC_CACHE["nc"] = _build_kernel()
    nc = _NC_CACHE["nc"]

    hidden = np.asarray(hidden)
    enc16 = np.ascontiguousarray(np.asarray(encoder_outputs).astype(np.float16))
    W16 = np.ascontiguousarray(np.asarray(W).astype(np.float16))
    in_maps = [
        {
            "enc": enc16[c],
            "hvec": np.ascontiguousarray(
                hidden[0, c][None, :].astype(np.float16)
            ),
            "W": W16,
        }
        for c in range(N_CORES)
    ]
    res = run_bass_kernel_spmd(nc, in_maps, core_ids=list(range(N_CORES)))
    return np.stack([res.results[c]["out"] for c in range(N_CORES)], axis=0).astype(
        np.float32
    )


# revision 10
# speedup vs baseline: 1.5290x; 1.0986x over previous
"""Trainium2 Bass kernel for the attention-scoring module:

    out[b, s] = softmax_s( (enc[b] @ W.T + bias) @ h[b] )

Math: the bias term contributes a constant per (b, :) row, which cancels in
the softmax, and the two contractions reassociate:

    energies[b, s] = enc[b, s, :] . v[b]   with   v[b] = h[b] @ W

Sharding: data-parallel over batch - one batch per NeuronCore (B == 8 cores).

This revision streams enc (and W, h) as float16: the softmax tolerates the
quantization (measured rel-l2 ~6e-4 on the harness inputs, vs the 2e-2
gate), and the serializing resource is the DMA bus, so halving the bytes
halves the stream time (16 MiB -> 8 MiB, ~46.6us -> ~23.3us of DMA busy).

At fp16 arrival rates (~364 ns per 128-row block) no single engine keeps up
with the dot products (the fused DVE multiply+row-sum runs at 1x,
~612 ns/block), so the 64 blocks are split across three paths:

  - 'D': DVE fused scalar_tensor_tensor multiply+row-sum  (~612 ns/block)
  - 'A': DVE tensor_tensor multiply in fp16 2x mode (~332 ns) + ACT
         Copy-activation with accum_out row-sum            (~810 ns/block)
  - 'G': GPSIMD tensor_tensor multiply (~1110 ns) + the same ACT reduce
         (the Pool engine cannot run the fused scalar_tensor_tensor, and
         XBAR-transposed loads for a PE path serialize against regular
         DMAs, so both alternatives lose).

Softmax is incremental: shift from the first SHIFT_C columns mid-stream,
staged exp+row-sum, PE transposes of the prob columns, and a short tail
(exp of the last columns + sum + reciprocal + scale + one output DMA).
"""

from contextlib import ExitStack

import numpy as np

import concourse.tile as tile
from concourse import bacc, mybir
from concourse import bass_isa
from concourse.bass_utils import run_bass_kernel_spmd
from concourse.masks import make_identity

B, S, H = 8, 8192, 512
N_CORES = 8
P = 128
N_COLS = S // P  # 64 energy columns, E[p, t] = energy(s = t*128 + p)
F32 = mybir.dt.float32
F16 = mybir.dt.float16
ALU = mybir.AluOpType
ACTF = mybir.ActivationFunctionType
AXX = mybir.AxisListType.X

SHIFT_C = 16  # softmax shift comes from the first 16 columns, mid-stream
EC = 32       # stage-1 exp/transpose boundary (32-aligned partition offsets)
MC = 62       # second exp stage covers cols EC..MC

# enc DMA grouping: big groups early, singles late so the tail only waits
# on one 128-row block.
DMA_GROUPS = [8] * 6 + [4] * 2 + [2] * 2 + [1] * 4

# Per-block engine costs (ns) for the offline greedy scheduler.
COST_DVE_FUSED = 612.0
COST_DVE_MULT = 332.0
COST_ACT_REDUCE = 810.0
COST_GP_MULT = 1110.0


def _sim_schedule(assign, dma_groups):
    """Small discrete-event model of the kernel schedule: in-order engine
    queues, DMA arrivals, cross-engine deps.  Returns estimated end time."""
    SEM = 30.0
    t = 1970.0 + 3.0 + 1456.0  # h + one W DMA
    arrivals = []
    for g in dma_groups:
        t += g * 364.0
        arrivals += [t + 900.0] * g
    v_ready = 5700.0
    free = {"D": v_ready, "A": v_ready, "G": v_ready}
    edone = [0.0] * N_COLS  # time E[:, b] is written
    negM = None
    rs1_done = 0.0

    def run(eng, ready, cost):
        s = max(free[eng], ready)
        free[eng] = s + cost
        return s + cost

    for b in range(N_COLS):
        p = assign[b]
        if p == "D":
            edone[b] = run("D", arrivals[b], 612.0)
        elif p == "A":
            m = run("D", arrivals[b], 332.0)
            edone[b] = run("A", m + SEM, 810.0)
        else:
            m = run("G", arrivals[b], 1110.0)
            edone[b] = run("A", m + SEM, 810.0)
        if b + 1 == SHIFT_C:
            mx = max(edone[:SHIFT_C]) + SEM
            r = run("D", mx, 80.0)
            g = run("G", r + SEM, 250.0)
            negM = run("D", g + SEM, 62.0)
        if b + 1 == EC:
            mx = max(max(edone[:EC]) + SEM, negM + SEM)
            rs1_done = run("A", mx, 400.0)
        if b + 1 == MC:
            mx = max(max(edone[:MC]) + SEM, negM + SEM)
            s2 = run("A", mx, 450.0)
            run("D", max(s2 + SEM, rs1_done + SEM), 62.0)
    # tail
    e_tail = run("A", max(edone[MC : N_COLS - 1]) + SEM, 210.0)
    e63 = run("A", max(edone[N_COLS - 1] + SEM, e_tail), 210.0)
    red = run("D", e63 + SEM, 75.0)
    allr = run("G", red + SEM, 250.0)
    rec = run("D", allr + SEM, 62.0)
    s1 = run("D", rec, 193.0)
    s2 = run("D", s1, 193.0)
    return s2 + SEM + 625.0 + 650.0 + 91.0 + 800.0


def _assign_engines():
    """Balanced base assignment improved by local search against the
    schedule model."""
    import random

    rng = random.Random(1234)
    # balanced seed: mostly D with G every ~3rd block and sparse A
    base = []
    for b in range(N_COLS):
        r = b % 16
        if r in (1, 4, 7, 10, 13):
            base.append("G")
        elif r == 14:
            base.append("A")
        else:
            base.append("D")
    base[N_COLS - 1] = "D"
    best, best_t = base[:], _sim_schedule(base, DMA_GROUPS)
    for _ in range(4000):
        cand = best[:]
        for _ in range(rng.randint(1, 2)):
            i = rng.randrange(N_COLS - 1)
            cand[i] = rng.choice("DAG".replace(cand[i], ""))
        t = _sim_schedule(cand, DMA_GROUPS)
        if t <= best_t:
            best, best_t = cand, t
    counts = {k: best.count(k) for k in "DAG"}
    return best


def _build_kernel():
    nc = bacc.Bacc("TRN2", target_bir_lowering=False, debug=False)
    enc = nc.dram_tensor("enc", [S, H], F16, kind="ExternalInput")
    hvec = nc.dram_tensor("hvec", [1, H], F16, kind="ExternalInput")
    Wmat = nc.dram_tensor("W", [H, H], F16, kind="ExternalInput")
    out = nc.dram_tensor("out", [S], F32, kind="ExternalOutput")

    engine_of = _assign_engines()

    with ExitStack() as ctx:
        tc = ctx.enter_context(tile.TileContext(nc))
        consts = ctx.enter_context(tc.tile_pool(name="consts", bufs=1))
        small = ctx.enter_context(tc.tile_pool(name="small", bufs=1))
        psum = ctx.enter_context(tc.tile_pool(name="psum", bufs=1, space="PSUM"))

        # Big flat SBUF regions (raw tensors: no pool-close drain and no
        # buffer recycling, so block compute carries no WAR waits).
        enc_t = ctx.enter_context(nc.sbuf_tensor("enc_all", [P, N_COLS, H], F16))
        enc_all = enc_t.ap()
        prod_t = ctx.enter_context(nc.sbuf_tensor("prods", [P, N_COLS, H], F16))
        prods = prod_t.ap()
        final_t = ctx.enter_context(nc.sbuf_tensor("final", [N_COLS, P], F32))
        final = final_t.ap()

        # Constants (identity first: the PE warm-up waits on it).
        identity = consts.tile([P, P], F32)
        make_identity(nc, identity[:])
        ones_row16 = consts.tile([1, P], F16)
        nc.gpsimd.memset(ones_row16[:], 1.0)
        one11 = consts.tile([1, 1], F32)
        nc.gpsimd.memset(one11[:], 1.0)

        # ---- input DMA queue: hvec, W (4 chunks), then enc groups ----
        hrow = small.tile([1, H], F16)
        nc.sync.dma_start(hrow[:], hvec.ap())
        W_sb = small.tile([P, 4, H], F16)
        nc.sync.dma_start(
            W_sb[:], Wmat.ap().rearrange("(c p) h -> p c h", c=4, p=P)
        )
        blk = 0
        for g in DMA_GROUPS:
            nc.sync.dma_start(
                enc_all[:, blk : blk + g, :],
                enc.ap()[blk * P : (blk + g) * P, :].rearrange(
                    "(c p) h -> p c h", c=g, p=P
                ),
            )
            blk += g

        # Trigger the ACT exp table load at t=0 instead of in the tail.
        dummy_act = small.tile([1, 1], F32)
        nc.scalar.activation(dummy_act[:], one11[:], ACTF.Exp, bias=0.0, scale=1.0)

        # PE p-state warm-up: keep the PE continuously busy until hrow
        # lands so the v matmuls run at a ramped clock.
        warm = psum.tile([P, P], F32, tag="pwarm")
        for i in range(7):
            nc.tensor.transpose(warm[:], identity[:], identity[:])

        # ---- v = h @ W, broadcast to all 128 partitions (fp16 pipeline) ----
        # Stage 1: hTb_c[m, n] = h[c*128+m] for all n (hrow-chunk stationary
        # x ones_row moving); 4 distinct PSUM tags so nothing ping-pongs.
        # Copies PSUM->SBUF alternate ACT/DVE to halve the serial chain.
        hT_sb = []
        for c in range(4):
            hT_ps = psum.tile([P, P], F32, tag=f"ph{c}", name=f"hT_ps{c}")
            nc.tensor.matmul(
                hT_ps[:],
                hrow[:1, c * P : (c + 1) * P],
                ones_row16[:],
                start=True,
                stop=True,
            )
            ht = small.tile([P, P], F16, tag=f"ht{c}", name=f"ht{c}")
            hT_sb.append(ht)
            if c % 2 == 0:
                nc.scalar.copy(ht[:], hT_ps[:])
            else:
                nc.vector.tensor_copy(ht[:], hT_ps[:])
        v_bc_ps = psum.tile([P, H], F32, tag="vbc")
        for c in range(4):
            nc.tensor.matmul(
                v_bc_ps[:],
                hT_sb[c][:],
                W_sb[:, c, :],
                start=(c == 0),
                stop=(c == 3),
            )
        # v copy split across ACT and DVE so the consumers start sooner.
        v_sb = small.tile([P, H], F16)
        nc.scalar.copy(v_sb[:, : H // 2], v_bc_ps[:, : H // 2])
        nc.vector.tensor_copy(v_sb[:, H // 2 :], v_bc_ps[:, H // 2 :])

        # ---- main loop: stream enc, dot products split across 3 engines ----
        E = small.tile([P, N_COLS], F32)
        E63z = small.tile([P, 1], F32)  # dedicated final column
        P_exp = small.tile([P, N_COLS + 1], F32)  # col 64 holds rs12
        rs1 = small.tile([P, 1], F32)
        negM_sb = small.tile([P, 1], F32)
        probsT_ps = psum.tile([EC, P], F32, tag="ph0")
        probsT23_ps = psum.tile([N_COLS - EC, P], F32, tag="vbc")

        def emit_shift_chain():
            m_col = small.tile([P, 1], F32)
            nc.vector.tensor_reduce(
                m_col[:], E[:, :SHIFT_C], axis=AXX, op=ALU.max
            )
            M_bc = small.tile([P, 1], F32)
            nc.gpsimd.partition_all_reduce(
                M_bc[:], m_col[:], P, bass_isa.ReduceOp.max
            )
            nc.vector.tensor_scalar_mul(negM_sb[:], M_bc[:], -1.0)

        def emit_stage1_chain():
            nc.scalar.activation(
                P_exp[:, :EC],
                E[:, :EC],
                ACTF.Exp,
                bias=negM_sb[:],
                scale=1.0,
                accum_out=rs1[:],
            )
            nc.tensor.transpose(probsT_ps[:], P_exp[:, :EC], identity[:])

        def emit_mid_chain():
            rs2 = small.tile([P, 1], F32)
            nc.scalar.activation(
                P_exp[:, EC:MC],
                E[:, EC:MC],
                ACTF.Exp,
                bias=negM_sb[:],
                scale=1.0,
                accum_out=rs2[:],
            )
            nc.vector.tensor_add(P_exp[:, N_COLS : N_COLS + 1], rs1[:], rs2[:])

        for t in range(N_COLS):
            ch = enc_all[:, t, :]
            eng = engine_of[t]
            acc = E63z[:] if t == N_COLS - 1 else E[:, t : t + 1]
            pr = prods[:, t, :]
            if eng == "D":
                nc.vector.scalar_tensor_tensor(
                    out=pr, in0=ch, scalar=1.0, in1=v_sb[:],
                    op0=ALU.bypass, op1=ALU.mult, accum_out=acc,
                )
            elif eng == "G":
                nc.gpsimd.tensor_tensor(pr, ch, v_sb[:], op=ALU.mult)
                nc.scalar.activation(
                    pr, pr, ACTF.Copy, bias=0.0, scale=1.0, accum_out=acc,
                )
            else:  # 'A': fp16 2x multiply on DVE, reduce on ACT
                nc.vector.tensor_tensor(pr, ch, v_sb[:], op=ALU.mult)
                nc.scalar.activation(
                    pr, pr, ACTF.Copy, bias=0.0, scale=1.0, accum_out=acc,
                )
            if t + 1 == SHIFT_C:
                emit_shift_chain()
            if t + 1 == EC:
                emit_stage1_chain()
            if t + 1 == MC:
                emit_mid_chain()

        # ---- softmax tail: columns MC..63 ----
        nc.scalar.activation(
            P_exp[:, MC : N_COLS - 1],
            E[:, MC : N_COLS - 1],
            ACTF.Exp,
            bias=negM_sb[:],
            scale=1.0,
        )
        nc.scalar.activation(
            P_exp[:, N_COLS - 1 : N_COLS],
            E63z[:],
            ACTF.Exp,
            bias=negM_sb[:],
            scale=1.0,
        )
        nc.tensor.transpose(probsT23_ps[:], P_exp[:, EC:N_COLS], identity[:])
        rs_tot = small.tile([P, 1], F32)
        nc.vector.tensor_reduce(
            rs_tot[:], P_exp[:, MC : N_COLS + 1], axis=AXX, op=ALU.add
        )
        S_bc = small.tile([P, 1], F32)
        nc.gpsimd.partition_all_reduce(S_bc[:], rs_tot[:], P, bass_isa.ReduceOp.add)
        SinvB = small.tile([N_COLS, 1], F32)
        nc.vector.reciprocal(SinvB[:], S_bc[:N_COLS, :])
        # scale straight out of PSUM (SinvB entries identical -> base-0 ok)
        nc.vector.tensor_scalar_mul(
            final[EC:, :], probsT23_ps[:], SinvB[: N_COLS - EC, :]
        )
        nc.vector.tensor_scalar_mul(final[:EC, :], probsT_ps[:], SinvB[:EC, :])
        nc.sync.dma_start(out.ap().rearrange("(t p) -> t p", p=P), final)

    nc.compile()
    return nc


_NC_CACHE = {}


def kernel(hidden, encoder_outputs, W, b):
    """Full (unsharded) inputs in, full output out; 8-core SPMD inside."""
    if "nc" not in _NC_CACHE:
        _NC_CACHE["nc"] = _build_kernel()
    nc = _NC_CACHE["nc"]

    hidden = np.asarray(hidden)
    enc16 = np.ascontiguousarray(np.asarray(encoder_outputs).astype(np.float16))
    W16 = np.ascontiguousarray(np.asarray(W).astype(np.float16))
    in_maps = [
        {
            "enc": enc16[c],
            "hvec": np.ascontiguousarray(
                hidden[0, c][None, :].astype(np.float16)
            ),
            "W": W16,
        }
        for c in range(N_CORES)
    ]
    res = run_bass_kernel_spmd(nc, in_maps, core_ids=list(range(N_CORES)))
    return np.stack([res.results[c]["out"] for c in range(N_CORES)], axis=0).astype(
        np.float32
    )
